# revision 4
# baseline (speedup 1.0000x reference)
"""Multi-head attention (B=4, S=2048, D=1024, H=16) on 8 TRN2 NeuronCores.

Sharding: core c <- batch c//2, heads 8*(c%2) .. 8*(c%2)+8 (Megatron-style:
Wq/Wk/Wv column-parallel, Wo row-parallel). No collectives: the two partial
outputs per batch are summed on the host (plus the bo bias).

Per-core kernel strategy:
  - q^T, k^T computed directly in [head_dim, seq] layout (out = W^T.T @ X^T),
    v computed in natural [seq, head_dim] layout with a ones column appended.
  - Scores computed transposed: ST[s_k, s_q] = k . q, so softmax exp is pure
    elementwise (no max subtraction needed: scores ~ N(0,1) after 1/8 scale,
    fp32 exp cannot overflow) and no on-chip transposes are needed anywhere.
  - ctx^T[c, s_q] accumulated as v_aug^T @ exp(ST); the ones column yields the
    softmax denominator l[s_q] as psum row 64 for free.
  - Normalization by 1/l folded in before the output projection.
  - All matmuls run as float32r (full PE rate at N>=256, ~1.5e-4 rel err).
"""
import sys

sys.path.insert(0, "/opt/trn_rl_repo")
import numpy as np

import concourse.bass as bass
import concourse.bacc as bacc
import concourse.mybir as mybir
import concourse.tile as tile
from concourse.bass_utils import run_bass_kernel_spmd

f32 = mybir.dt.float32
f32r = mybir.dt.float32r
EXP = mybir.ActivationFunctionType.Exp

S = 2048          # sequence length
D = 1024          # model dim
HC = 8            # heads per core
DK = 64           # head dim
JC = HC * DK      # per-core projection width (512)
SCALE = 0.125     # 1/sqrt(DK)
N_CORES = 8


def build_nc():
    nc = bacc.Bacc(None, target_bir_lowering=False, debug=False)

    qt = nc.dram_tensor("qt", [D, S], f32r, kind="ExternalInput")
    kt = nc.dram_tensor("kt", [D, S], f32r, kind="ExternalInput")
    vt = nc.dram_tensor("vt", [D, S], f32r, kind="ExternalInput")
    wqt = nc.dram_tensor("wqt", [D, JC], f32r, kind="ExternalInput")
    wkt = nc.dram_tensor("wkt", [D, JC], f32r, kind="ExternalInput")
    wvt = nc.dram_tensor("wvt", [D, JC], f32r, kind="ExternalInput")
    wot = nc.dram_tensor("wot", [JC, D], f32r, kind="ExternalInput")
    bq = nc.dram_tensor("bq", [128, 4], f32, kind="ExternalInput")
    bk = nc.dram_tensor("bk", [128, 4], f32, kind="ExternalInput")
    bvb = nc.dram_tensor("bvb", [128, JC], f32, kind="ExternalInput")
    out = nc.dram_tensor("out", [S, D], f32, kind="ExternalOutput")

    with tile.TileContext(nc) as tc:
        with (
            tc.tile_pool(name="big", bufs=1) as big,
            tc.tile_pool(name="work", bufs=3) as work,
        ):
            # persistent activations
            qT_sb = big.tile([128, 4, S], f32r)          # [p, jt, s]: q[s, jt*128+p]
            kT_sb = big.tile([128, 4, S], f32r)
            v_sb = big.tile([128, 16, HC, DK + 1], f32r)  # [p, st, h, c]: v[st*128+p, 64h+c]
            ctxn_sb = big.tile([128, 4, S], f32r)         # [p, pair, s]: normalized ctx^T
            bq_sb = big.tile([128, 4], f32)
            bk_sb = big.tile([128, 4], f32)
            bvb_sb = big.tile([128, JC], f32)

            nc.sync.dma_start(bq_sb[:], bq[:])
            nc.sync.dma_start(bk_sb[:], bk[:])
            nc.sync.dma_start(bvb_sb[:], bvb[:])
            nc.vector.memset(v_sb[:, :, :, DK].bitcast(f32), 1.0)     # ones column

            # ---------------- Stage 1: projections ----------------
            SC = 256  # seq chunk for q/k projections
            for x_dram, w_dram, o_sb, b_sb in (
                (qt, wqt, qT_sb, bq_sb),
                (kt, wkt, kT_sb, bk_sb),
            ):
                with (
                    tc.tile_pool(name="w1", bufs=1) as wp,
                    tc.tile_pool(name="ps1", bufs=2, space="PSUM") as pp,
                ):
                    w_sb = wp.tile([128, 8, JC], f32r, tag="w")
                    nc.sync.dma_start(
                        w_sb[:], w_dram.rearrange("(kt p) j -> p kt j", p=128)
                    )
                    for sc in range(S // SC):
                        xq = work.tile([128, 8, SC], f32r, tag="x")
                        nc.sync.dma_start(
                            xq[:],
                            x_dram[:, sc * SC:(sc + 1) * SC].rearrange(
                                "(kt p) s -> p kt s", p=128
                            ),
                        )
                        for jt in range(4):
                            ps = pp.tile([128, SC], f32, tag="proj")
                            for ktile in range(8):
                                nc.tensor.matmul(
                                    ps[:],
                                    w_sb[:, ktile, jt * 128:(jt + 1) * 128],
                                    xq[:, ktile, :],
                                    start=(ktile == 0),
                                    stop=(ktile == 7),
                                )
                            nc.vector.tensor_scalar_add(
                                o_sb[:, jt, sc * SC:(sc + 1) * SC],
                                ps[:],
                                b_sb[:, jt:jt + 1],
                            )

            with (
                tc.tile_pool(name="w1", bufs=1) as wp,
                tc.tile_pool(name="ps1", bufs=2, space="PSUM") as pp,
            ):
                wv_sb = wp.tile([128, 8, JC], f32r, tag="w")
                nc.sync.dma_start(
                    wv_sb[:], wvt.rearrange("(kt p) j -> p kt j", p=128)
                )
                for st in range(16):
                    xv = work.tile([128, 8, 128], f32r, tag="x")
                    nc.sync.dma_start(
                        xv[:],
                        vt[:, st * 128:(st + 1) * 128].rearrange(
                            "(kt p) s -> p kt s", p=128
                        ),
                    )
                    ps = pp.tile([128, JC], f32, tag="proj")
                    for ktile in range(8):
                        nc.tensor.matmul(
                            ps[:],
                            xv[:, ktile, :],
                            wv_sb[:, ktile, :],
                            start=(ktile == 0),
                            stop=(ktile == 7),
                        )
                    nc.vector.tensor_add(
                        v_sb[:, st, :, 0:DK],
                        ps[:].rearrange("p (h c) -> p h c", h=HC),
                        bvb_sb[:].rearrange("p (h c) -> p h c", h=HC),
                    )

            # ---------------- Stage 2: attention ----------------
            with (
                tc.tile_pool(name="ps2", bufs=2, space="PSUM") as pp2,
                tc.tile_pool(name="att", bufs=2) as att,
            ):
                for p in range(4):            # head pairs: hA=2p (part 0-63), hB=2p+1 (64-127)
                    hA, hB = 2 * p, 2 * p + 1
                    for sq in range(4):       # s_q chunks of 512
                        ctxA = pp2.tile([DK + 1, 512], f32, tag="ctxA")
                        ctxB = pp2.tile([DK + 1, 512], f32, tag="ctxB")
                        for k in range(16):   # s_k tiles of 128
                            stA = pp2.tile([128, 512], f32, tag="stA")
                            stB = pp2.tile([128, 512], f32, tag="stB")
                            nc.tensor.matmul(
                                stA[:],
                                kT_sb[0:64, p, k * 128:(k + 1) * 128],
                                qT_sb[0:64, p, sq * 512:(sq + 1) * 512],
                                start=True, stop=True,
                            )
                            nc.tensor.matmul(
                                stB[:],
                                kT_sb[64:128, p, k * 128:(k + 1) * 128],
                                qT_sb[64:128, p, sq * 512:(sq + 1) * 512],
                                start=True, stop=True,
                            )
                            ptA = att.tile([128, 512], f32r, tag="ptA")
                            ptB = att.tile([128, 512], f32r, tag="ptB")
                            nc.scalar.activation(ptA[:], stA[:], EXP, scale=SCALE)
                            nc.scalar.activation(ptB[:], stB[:], EXP, scale=SCALE)
                            nc.tensor.matmul(
                                ctxA[:],
                                v_sb[:, k, hA, :],
                                ptA[:],
                                start=(k == 0), stop=(k == 15),
                            )
                            nc.tensor.matmul(
                                ctxB[:],
                                v_sb[:, k, hB, :],
                                ptB[:],
                                start=(k == 0), stop=(k == 15),
                            )
                        # softmax denominators are psum row 64; normalize
                        rA = att.tile([1, 512], f32, tag="rA")
                        rB = att.tile([1, 512], f32, tag="rB")
                        nc.vector.reciprocal(rA[:], ctxA[DK:DK + 1, :])
                        nc.vector.reciprocal(rB[:], ctxB[DK:DK + 1, :])
                        rAb = att.tile([64, 512], f32, tag="rAb")
                        rBb = att.tile([64, 512], f32, tag="rBb")
                        nc.gpsimd.partition_broadcast(rAb[:], rA[:])
                        nc.gpsimd.partition_broadcast(rBb[:], rB[:])
                        nc.vector.tensor_mul(
                            ctxn_sb[0:64, p, sq * 512:(sq + 1) * 512],
                            ctxA[0:DK, :], rAb[:],
                        )
                        nc.vector.tensor_mul(
                            ctxn_sb[64:128, p, sq * 512:(sq + 1) * 512],
                            ctxB[0:DK, :], rBb[:],
                        )

            # ---------------- Stage 3: output projection ----------------
            with (
                tc.tile_pool(name="w3", bufs=1) as wp3,
                tc.tile_pool(name="ps3", bufs=2, space="PSUM") as pp3,
            ):
                wot_sb = wp3.tile([128, 4, D], f32r)
                nc.sync.dma_start(
                    wot_sb[:], wot.rearrange("(kt p) j -> p kt j", p=128)
                )
                for sq2 in range(16):
                    for n in range(2):
                        ps = pp3.tile([128, 512], f32, tag="o")
                        for p in range(4):
                            nc.tensor.matmul(
                                ps[:],
                                ctxn_sb[:, p, sq2 * 128:(sq2 + 1) * 128],
                                wot_sb[:, p, n * 512:(n + 1) * 512],
                                start=(p == 0), stop=(p == 3),
                            )
                        ob = work.tile([128, 512], f32, tag="ob")
                        nc.vector.tensor_copy(ob[:], ps[:])
                        nc.sync.dma_start(
                            out[sq2 * 128:(sq2 + 1) * 128, n * 512:(n + 1) * 512],
                            ob[:],
                        )

    nc.compile()
    return nc


_NC = None


def _get_nc():
    global _NC
    if _NC is None:
        _NC = build_nc()
    return _NC


def make_in_maps(Q, K, V, Wq, bq, Wk, bk, Wv, bv, Wo, bo):
    asf = lambda x: np.ascontiguousarray(np.asarray(x, dtype=np.float32))
    in_maps = []
    for c in range(N_CORES):
        b = c // 2
        j0 = JC * (c % 2)
        jsl = slice(j0, j0 + JC)
        in_maps.append({
            "qt": asf(np.asarray(Q)[b].T),
            "kt": asf(np.asarray(K)[b].T),
            "vt": asf(np.asarray(V)[b].T),
            "wqt": asf(np.asarray(Wq)[jsl].T),
            "wkt": asf(np.asarray(Wk)[jsl].T),
            "wvt": asf(np.asarray(Wv)[jsl].T),
            "wot": asf(np.asarray(Wo)[:, jsl].T),
            "bq": asf(np.asarray(bq)[jsl].reshape(4, 128).T),
            "bk": asf(np.asarray(bk)[jsl].reshape(4, 128).T),
            "bvb": asf(np.broadcast_to(np.asarray(bv)[jsl], (128, JC))),
        })
    return in_maps


def kernel(Q, K, V, Wq, bq, Wk, bk, Wv, bv, Wo, bo, _trace=False, _trace_kwargs=None):
    nc = _get_nc()
    in_maps = make_in_maps(Q, K, V, Wq, bq, Wk, bk, Wv, bv, Wo, bo)
    res = run_bass_kernel_spmd(
        nc, in_maps, core_ids=list(range(N_CORES)),
        trace=_trace, **(_trace_kwargs or {}),
    )
    parts = [res.results[c]["out"] for c in range(N_CORES)]
    bo_np = np.asarray(bo, dtype=np.float32)
    O = np.stack([parts[2 * b] + parts[2 * b + 1] + bo_np for b in range(4)])
    kernel.last_results = res
    return O.astype(np.float32)
